# revision 21
# baseline (speedup 1.0000x reference)
"""Causal self-attention (B=2, T=2048, d_model=1024, H=16) on 8 TRN2 NeuronCores.

Sharding: core c handles batch b = c//4 and head group g = c%4 (heads 4g..4g+3).
Each core computes QKV projection for its heads, causal attention, and a partial
output projection y_partial = attn_out @ Wo[g*256:(g+1)*256, :]. The host sums
the 4 partials per batch (the tensor-parallel all-reduce, done on host).

All device compute is bf16 (inputs cast host-side; PSUM accumulation stays
f32; the y partials are bf16 and summed in f32 on the host), which halves
HBM traffic and removes the fp32r narrow-matmul penalty on the PE.

Per-core structure (one rep):
  qk(pair0) -> V(all heads) -> attention(pair0) -> qk(pair1)
  -> [attention(pair1, chunk c) -> out-proj(t-blocks of chunk c)] for c in 0..3
Attention runs on 512-query chunks with both heads of a pair sharing one
[128, 1024] S PSUM tile so each exp is one wide ACT instruction.  The PV
matmul uses V' = [V | 1] so its PSUM row 64 is the softmax denominator;
reciprocal is taken in-place on that row and broadcast via a small bf16 DMA.
The emission order lets the tile scheduler fill PE gaps during the ACT-paced
attention with qk/out-proj matmuls.
"""
import sys

sys.path.insert(0, "/opt/trn_rl_repo")

import numpy as np
import ml_dtypes

B, T, C = 2, 2048, 1024
NH_TOT = 16
HD = 64
NH = 4          # heads per core
CO = NH * HD    # 256 channels per core
NCORES = 8
SCALE = 1.0 / 32.0  # d_model ** -0.5

_compiled = None


def _build(nrep=1, trace_sim=False):
    import concourse.bass as bass  # noqa: F401
    import concourse.mybir as mybir
    import concourse.tile as tile
    from concourse import bacc

    F32 = mybir.dt.float32
    BF16 = mybir.dt.bfloat16
    MULT = mybir.AluOpType.mult
    EXP = mybir.ActivationFunctionType.Exp

    nc = bacc.Bacc("TRN2", target_bir_lowering=False)

    xT = nc.declare_dram_parameter("xT", [C, T], BF16, isOutput=False)
    wq = nc.declare_dram_parameter("wq", [C, CO], BF16, isOutput=False)
    wk = nc.declare_dram_parameter("wk", [C, CO], BF16, isOutput=False)
    wv = nc.declare_dram_parameter("wv", [C, CO], BF16, isOutput=False)
    wo = nc.declare_dram_parameter("wo", [CO, C], BF16, isOutput=False)
    mask = nc.declare_dram_parameter("mask", [128, 128], BF16, isOutput=False)
    y = nc.declare_dram_parameter("y", [T, C], BF16, isOutput=True)

    xT_t = xT.rearrange("(o p) t -> p o t", p=128)   # [128, 8, 2048]
    wq_t = wq.rearrange("(o p) m -> p o m", p=128)   # [128, 8, 256]
    wk_t = wk.rearrange("(o p) m -> p o m", p=128)
    wv_t = wv.rearrange("(o p) m -> p o m", p=128)
    wo_t = wo.rearrange("(o p) m -> p o m", p=128)   # [128, 2, 1024]

    with tile.TileContext(nc, trace_sim=trace_sim) as tc:
        with (
            nc.allow_low_precision(reason="bf16 matmul pipeline"),
            tc.tile_pool(name="wpool", bufs=1) as wpool,
            tc.tile_pool(name="qkvpool", bufs=2) as qkvpool,
            tc.tile_pool(name="psS", bufs=2, space="PSUM") as psS,
            tc.tile_pool(name="psP", bufs=2, space="PSUM") as psP,
            tc.tile_pool(name="psA", bufs=2, space="PSUM") as psA,
            tc.tile_pool(name="xpool", bufs=2) as xpool,
            tc.tile_pool(name="etpool", bufs=6) as etpool,
            tc.tile_pool(name="rcpool", bufs=4) as rcpool,
            tc.tile_pool(name="bcpool", bufs=4) as bcpool,
            tc.tile_pool(name="ypool", bufs=3) as ypool,
        ):
            wq_sb = wpool.tile([128, 8, CO], BF16, tag="wq")
            wk_sb = wpool.tile([128, 8, CO], BF16, tag="wk")
            wv_sb = wpool.tile([128, 8, CO], BF16, tag="wv")
            wo_sb = wpool.tile([128, 2, C], BF16, tag="wo")
            mask_sb = wpool.tile([128, 128], BF16, tag="mask")
            nc.sync.dma_start(wq_sb[:], wq_t[:])
            nc.sync.dma_start(wk_sb[:], wk_t[:])
            nc.sync.dma_start(wv_sb[:], wv_t[:])
            nc.sync.dma_start(wo_sb[:], wo_t[:])
            nc.sync.dma_start(mask_sb[:], mask[:])

            def emit_x_dma(dst):
                for th in range(2):
                    for kc in range(8):
                        nc.sync.dma_start(
                            dst[:, kc, th * 1024:(th + 1) * 1024],
                            xT_t[:, kc, th * 1024:(th + 1) * 1024],
                        )

            def emit_qk(pair, xT_sb, qt, kt):
                # q/k for the pair's 2 heads: [128 chans, T] each, in
                # per-pair tiles (no false WAR against the other pair's
                # attention reads)
                for w_sb, dst in ((wq_sb, qt), (wk_sb, kt)):
                    for t4 in range(4):
                        ps = psA.tile([128, 512], F32, tag="aux",
                                      name="ps_qk")
                        for kc in range(8):
                            nc.tensor.matmul(
                                ps[:],
                                w_sb[:, kc, pair * 128:(pair + 1) * 128],
                                xT_sb[:, kc, t4 * 512:(t4 + 1) * 512],
                                start=(kc == 0),
                                stop=(kc == 7),
                            )
                        nc.vector.tensor_copy(
                            dst[:, t4 * 512:(t4 + 1) * 512], ps[:]
                        )

            def alloc_qk(pair):
                qt = qkvpool.tile([128, T], BF16, tag=f"qT{pair}",
                                  name=f"qT{pair}")
                kt = qkvpool.tile([128, T], BF16, tag=f"kT{pair}",
                                  name=f"kT{pair}")
                return qt, kt

            # Software pipeline: x DMA and the next rep's pair-0 q/k are
            # issued one rep ahead so PE never starves at rep boundaries.
            xT_cur = xpool.tile([128, 8, T], BF16, tag="xT", name="xT0")
            emit_x_dma(xT_cur)
            qk0_cur = alloc_qk(0)
            emit_qk(0, xT_cur, *qk0_cur)

            for _rep in range(nrep):
                xT_sb = xT_cur
                qTs, kTs = {}, {}
                qTs[0], kTs[0] = qk0_cur
                # V' per (t-block, head): 64 cols of V then a ones column.
                # Split into 4 t-quarters so attention chunk c only gates on
                # the quarters it reads (per-tensor sems), not the whole V
                # phase.
                vp_sb = [
                    qkvpool.tile([128, 4, NH, HD + 1], BF16, tag=f"vp{q}",
                                 name=f"vp{q}")
                    for q in range(4)
                ]
                oT_sb = qkvpool.tile([128, 2, T], BF16, tag="oT", name="oT")
                for q in range(4):
                    nc.vector.memset(vp_sb[q][:, :, :, HD], 1.0)

                def emit_v(q):
                    # V in [t on partitions, head channels] layout, all 4
                    # heads, t-quarter q (t-blocks 4q..4q+3)
                    for tb in range(4 * q, 4 * q + 4):
                        ps = psA.tile([128, 512], F32, tag="aux", name="ps_v")
                        for kc in range(8):
                            nc.tensor.matmul(
                                ps[:, 0:CO],
                                xT_sb[:, kc, tb * 128:(tb + 1) * 128],
                                wv_sb[:, kc, :],
                                start=(kc == 0),
                                stop=(kc == 7),
                            )
                        nc.vector.tensor_copy(
                            vp_sb[tb // 4][:, tb % 4, :, 0:HD],
                            ps[:, 0:CO].rearrange("p (h d) -> p h d", h=NH),
                        )

                def attention_chunk(pair, c):
                    # queries [512c, 512c+512), key blocks jb = 0..4c+3
                    jb_last = 4 * c + 3
                    pos = [
                        psP.tile([65, 512], F32, tag="pos", name="pos")
                        for _ in range(2)
                    ]

                    def emit_s(jb):
                        off = 128 * max(0, jb - 4 * c)
                        ps_s = psS.tile([128, 1024], F32, tag="S", name="psS")
                        for hi in range(2):
                            nc.tensor.matmul(
                                ps_s[:, 512 * hi + off:512 * hi + 512],
                                kT_sb[64 * hi:64 * hi + 64, pair,
                                      jb * 128:(jb + 1) * 128],
                                qT_sb[64 * hi:64 * hi + 64, pair,
                                      c * 512 + off:(c + 1) * 512],
                                start=True,
                                stop=True,
                            )
                        et = etpool.tile([128, 1024], BF16, tag="et",
                                         name="et")
                        if off == 0:
                            nc.scalar.activation(et[:], ps_s[:], EXP,
                                                 scale=SCALE)
                        else:
                            # one strided activation covering both heads'
                            # valid spans [off:512) and [512+off:1024)
                            nc.scalar.activation(
                                et.rearrange("p (h w) -> p h w", h=2)[
                                    :, :, off:512],
                                ps_s.rearrange("p (h w) -> p h w", h=2)[
                                    :, :, off:512],
                                EXP, scale=SCALE,
                            )
                        if jb >= 4 * c:  # diagonal block: causal mask
                            for hi in range(2):
                                sl = slice(512 * hi + off, 512 * hi + off + 128)
                                nc.vector.tensor_tensor(
                                    et[:, sl], et[:, sl], mask_sb[:], MULT
                                )
                        return et, off

                    def emit_pv(jb, et, off):
                        for hi in range(2):
                            nc.tensor.matmul(
                                pos[hi][:, off:512],
                                vp_sb[jb // 4][:, jb % 4, 2 * pair + hi, :],
                                et[:, 512 * hi + off:512 * hi + 512],
                                start=(jb == 0),
                                stop=(jb == jb_last),
                            )

                    pending = emit_s(0)
                    for jb in range(jb_last + 1):
                        nxt = emit_s(jb + 1) if jb < jb_last else None
                        emit_pv(jb, *pending)
                        pending = nxt

                    # normalize: copy numerators out and take the recip of the
                    # sums row immediately (releases the pos PSUM for the next
                    # chunk's PV), then broadcast and scale oT in-place (all
                    # bf16 -> 2x DVE mode)
                    isl = slice(c * 512, (c + 1) * 512)
                    for hi in range(2):
                        o_sl = oT_sb[64 * hi:64 * hi + 64, pair, isl]
                        nc.vector.tensor_copy(o_sl, pos[hi][0:64, :])
                        rc = rcpool.tile([128, 512], BF16, tag="rc", name="rc")
                        nc.vector.reciprocal(rc[64:65, :], pos[hi][64:65, :])
                        bc = bcpool.tile([128, 512], BF16, tag="bc", name="bc")
                        # ACT-queue DMA: keeps the tiny broadcast off the SP
                        # queue (which carries the next rep's x prefetch)
                        nc.scalar.dma_start(
                            bc[0:64, :],
                            rc[64:65, None, :].to_broadcast([1, 64, 512]),
                        )
                        nc.vector.tensor_tensor(o_sl, o_sl, bc[0:64, :], MULT)

                def outproj(c):
                    # t-blocks 4c..4c+3; one y DMA per 2 t-blocks
                    for g in range(2):
                        y2 = ypool.tile([128, 2, C], BF16, tag="yt", name="y2")
                        for sub in range(2):
                            tb = 4 * c + 2 * g + sub
                            for nk in range(2):
                                py = psA.tile([128, 512], F32, tag="aux",
                                              name="py")
                                for pr in range(2):
                                    nc.tensor.matmul(
                                        py[:],
                                        oT_sb[:, pr, tb * 128:(tb + 1) * 128],
                                        wo_sb[:, pr, nk * 512:(nk + 1) * 512],
                                        start=(pr == 0),
                                        stop=(pr == 1),
                                    )
                                dst = y2[:, sub, nk * 512:(nk + 1) * 512]
                                nc.vector.tensor_copy(dst, py[:])
                        tb2 = 2 * c + g
                        # Pool SWDGE queue: y stores don't block SP's x
                        # prefetch or the ACT queue's bc broadcasts
                        nc.gpsimd.dma_start(
                            y[tb2 * 256:(tb2 + 1) * 256, :].rearrange(
                                "(b p) c -> p b c", p=128
                            ),
                            y2[:],
                        )

                if _rep + 1 < nrep:
                    xT_cur = xpool.tile([128, 8, T], BF16, tag="xT",
                                        name="xTn")
                    emit_x_dma(xT_cur)
                for c in range(4):
                    emit_v(c)
                    attention_chunk(0, c)
                emit_qk(1, xT_sb, qT_sb, kT_sb)
                for c in range(3):
                    attention_chunk(1, c)
                    outproj(c)
                attention_chunk(1, 3)
                if _rep + 1 < nrep:
                    # next rep's pair-0 q/k ahead of the final out-proj: PE
                    # filler for the last norm-chain wait
                    qT_cur = qkvpool.tile([128, 2, T], BF16, tag="qT",
                                          name="qTn")
                    kT_cur = qkvpool.tile([128, 2, T], BF16, tag="kT",
                                          name="kTn")
                    emit_qk(0, xT_cur, qT_cur, kT_cur)
                outproj(3)

    nc.compile()
    return nc


def _get_nc():
    global _compiled
    if _compiled is None:
        _compiled = _build()
    return _compiled


class _Runner:
    """Compiled PJRT executor for the SPMD kernel, reusable across calls."""

    def __init__(self, nc):
        import jax
        import concourse.mybir as mybir
        from concourse import bass2jax
        from jax.experimental.shard_map import shard_map
        from jax.sharding import Mesh, PartitionSpec

        self.jax = jax
        self.nc = nc
        bass2jax.install_neuronx_cc_hook()

        partition_name = (
            nc.partition_id_tensor.name if nc.partition_id_tensor else None
        )
        in_names, out_names, out_avals, zero_outs = [], [], [], []
        for alloc in nc.m.functions[0].allocations:
            if not isinstance(alloc, mybir.MemoryLocationSet):
                continue
            name = alloc.memorylocations[0].name
            if alloc.kind == "ExternalInput":
                if name != partition_name:
                    in_names.append(name)
            elif alloc.kind == "ExternalOutput":
                out_names.append(name)
                shape = tuple(alloc.tensor_shape)
                dtype = mybir.dt.np(alloc.dtype)
                out_avals.append(jax.core.ShapedArray(shape, dtype))
                zero_outs.append(np.zeros(shape, dtype))
        self.in_names = in_names
        self.out_names = out_names
        self.out_avals = out_avals
        self.zero_outs = zero_outs
        all_names = tuple(in_names + out_names)

        if partition_name is not None:
            all_names = all_names + (partition_name,)

        def _body(*args):
            operands = list(args)
            if partition_name is not None:
                operands.append(bass2jax.partition_id_tensor())
            outs = bass2jax._bass_exec_p.bind(
                *operands,
                out_avals=tuple(out_avals),
                in_names=all_names,
                out_names=tuple(out_names),
                lowering_input_output_aliases=(),
                sim_require_finite=True,
                sim_require_nnan=True,
                nc=nc,
            )
            return tuple(outs)

        devices = jax.devices()[:NCORES]
        assert len(devices) == NCORES
        mesh = Mesh(np.asarray(devices), ("core",))
        self._sharding = jax.sharding.NamedSharding(mesh, PartitionSpec("core"))
        n_args = len(in_names) + len(out_names)
        self.fn = jax.jit(
            shard_map(
                _body,
                mesh=mesh,
                in_specs=(PartitionSpec("core"),) * n_args,
                out_specs=(PartitionSpec("core"),) * len(out_names),
                check_rep=False,
            ),
            keep_unused=True,
        )

    def device_args(self, in_maps):
        args = [
            np.concatenate([np.asarray(m[name]) for m in in_maps], axis=0)
            for name in self.in_names
        ]
        args += [
            np.zeros((NCORES * z.shape[0], *z.shape[1:]), z.dtype)
            for z in self.zero_outs
        ]
        return [self.jax.device_put(a, self._sharding) for a in args]

    def run_device(self, dev_args):
        return self.fn(*dev_args)

    def run(self, in_maps):
        out_arrs = self.fn(*self.device_args(in_maps))
        return [
            {
                name: np.asarray(out_arrs[i]).reshape(
                    NCORES, *self.out_avals[i].shape
                )[c]
                for i, name in enumerate(self.out_names)
            }
            for c in range(NCORES)
        ]


_runner = None


def _get_runner():
    global _runner
    if _runner is None:
        _runner = _Runner(_get_nc())
    return _runner


def make_in_maps(x, Wqkv, Wo):
    bf16 = ml_dtypes.bfloat16
    x = np.asarray(x, dtype=np.float32)
    Wqkv = np.asarray(Wqkv, dtype=np.float32).astype(bf16)
    Wo = np.asarray(Wo, dtype=np.float32).astype(bf16)
    mask = np.triu(np.ones((128, 128), dtype=np.float32)).astype(bf16)
    in_maps = []
    for c in range(NCORES):
        b, g = c // 4, c % 4
        in_maps.append({
            "xT": np.ascontiguousarray(x[b].T.astype(bf16)),
            "wq": np.ascontiguousarray(Wqkv[:, g * CO:(g + 1) * CO]),
            "wk": np.ascontiguousarray(Wqkv[:, C + g * CO:C + (g + 1) * CO]),
            "wv": np.ascontiguousarray(Wqkv[:, 2 * C + g * CO:2 * C + (g + 1) * CO]),
            "wo": np.ascontiguousarray(Wo[g * CO:(g + 1) * CO, :]),
            "mask": mask,
        })
    return in_maps


def gather_output(results):
    y = np.zeros((B, T, C), dtype=np.float32)
    for c in range(NCORES):
        y[c // 4] += np.asarray(results[c]["y"]).astype(np.float32)
    return y


def kernel(x, Wqkv, Wo):
    runner = _get_runner()
    in_maps = make_in_maps(x, Wqkv, Wo)
    return gather_output(runner.run(in_maps))


# revision 26
# speedup vs baseline: 1.4561x; 1.4561x over previous
"""Causal self-attention (B=2, T=2048, d_model=1024, H=16) on 8 TRN2 NeuronCores.

Sharding: core c handles batch b = c//4 and head group g = c%4 (heads 4g..4g+3).
Each core computes QKV projection for its heads, causal attention, and a partial
output projection y_partial = attn_out @ Wo[g*256:(g+1)*256, :]. The host sums
the 4 partials per batch (the tensor-parallel all-reduce, done on host).

All device compute is bf16 (inputs cast host-side; PSUM accumulation stays
f32; the y partials are bf16 and summed in f32 on the host), which halves
HBM traffic and removes the fp32r narrow-matmul penalty on the PE.

Per-core structure (one rep):
  qk(pair0) -> V(all heads) -> attention(pair0) -> qk(pair1)
  -> [attention(pair1, chunk c) -> out-proj(t-blocks of chunk c)] for c in 0..3
Attention runs on 512-query chunks with both heads of a pair sharing one
[128, 1024] S PSUM tile so each exp is one wide ACT instruction.  The PV
matmul uses V' = [V | 1] so its PSUM row 64 is the softmax denominator;
reciprocal is taken in-place on that row and broadcast via a small bf16 DMA.
The emission order lets the tile scheduler fill PE gaps during the ACT-paced
attention with qk/out-proj matmuls.
"""
import sys

sys.path.insert(0, "/opt/trn_rl_repo")

import numpy as np
import ml_dtypes

B, T, C = 2, 2048, 1024
NH_TOT = 16
HD = 64
NH = 4          # heads per core
CO = NH * HD    # 256 channels per core
NCORES = 8
SCALE = 1.0 / 32.0  # d_model ** -0.5

_compiled = None


def _build(nrep=1, trace_sim=False):
    import concourse.bass as bass  # noqa: F401
    import concourse.mybir as mybir
    import concourse.tile as tile
    from concourse import bacc

    F32 = mybir.dt.float32
    BF16 = mybir.dt.bfloat16
    MULT = mybir.AluOpType.mult
    EXP = mybir.ActivationFunctionType.Exp

    nc = bacc.Bacc("TRN2", target_bir_lowering=False)

    xT = nc.declare_dram_parameter("xT", [C, T], BF16, isOutput=False)
    wq = nc.declare_dram_parameter("wq", [C, CO], BF16, isOutput=False)
    wk = nc.declare_dram_parameter("wk", [C, CO], BF16, isOutput=False)
    wv = nc.declare_dram_parameter("wv", [C, CO], BF16, isOutput=False)
    wo = nc.declare_dram_parameter("wo", [CO, C], BF16, isOutput=False)
    mask = nc.declare_dram_parameter("mask", [128, 128], BF16, isOutput=False)
    y = nc.declare_dram_parameter("y", [T, C], BF16, isOutput=True)

    xT_t = xT.rearrange("(o p) t -> p o t", p=128)   # [128, 8, 2048]
    wq_t = wq.rearrange("(o p) m -> p o m", p=128)   # [128, 8, 256]
    wk_t = wk.rearrange("(o p) m -> p o m", p=128)
    wv_t = wv.rearrange("(o p) m -> p o m", p=128)
    wo_t = wo.rearrange("(o p) m -> p o m", p=128)   # [128, 2, 1024]

    with tile.TileContext(nc, trace_sim=trace_sim) as tc:
        with (
            nc.allow_low_precision(reason="bf16 matmul pipeline"),
            tc.tile_pool(name="wpool", bufs=1) as wpool,
            tc.tile_pool(name="qkvpool", bufs=2) as qkvpool,
            tc.tile_pool(name="psS", bufs=2, space="PSUM") as psS,
            tc.tile_pool(name="psP", bufs=2, space="PSUM") as psP,
            tc.tile_pool(name="psA", bufs=2, space="PSUM") as psA,
            tc.tile_pool(name="xpool", bufs=2) as xpool,
            tc.tile_pool(name="etpool", bufs=6) as etpool,
            tc.tile_pool(name="rcpool", bufs=4) as rcpool,
            tc.tile_pool(name="bcpool", bufs=4) as bcpool,
            tc.tile_pool(name="ypool", bufs=3) as ypool,
        ):
            wq_sb = wpool.tile([128, 8, CO], BF16, tag="wq")
            wk_sb = wpool.tile([128, 8, CO], BF16, tag="wk")
            wv_sb = wpool.tile([128, 8, CO], BF16, tag="wv")
            wo_sb = wpool.tile([128, 2, C], BF16, tag="wo")
            mask_sb = wpool.tile([128, 128], BF16, tag="mask")
            nc.sync.dma_start(wq_sb[:], wq_t[:])
            nc.sync.dma_start(wk_sb[:], wk_t[:])
            nc.sync.dma_start(wv_sb[:], wv_t[:])
            nc.sync.dma_start(wo_sb[:], wo_t[:])
            nc.sync.dma_start(mask_sb[:], mask[:])

            def emit_x_dma(dst):
                for th in range(2):
                    for kc in range(8):
                        nc.sync.dma_start(
                            dst[:, kc, th * 1024:(th + 1) * 1024],
                            xT_t[:, kc, th * 1024:(th + 1) * 1024],
                        )

            def emit_qk(pair, xT_sb, qt, kt):
                # q/k for the pair's 2 heads: [128 chans, T] each, in
                # per-pair tiles (no false WAR against the other pair's
                # attention reads)
                for w_sb, dst in ((wq_sb, qt), (wk_sb, kt)):
                    for t4 in range(4):
                        ps = psA.tile([128, 512], F32, tag="aux",
                                      name="ps_qk")
                        for kc in range(8):
                            nc.tensor.matmul(
                                ps[:],
                                w_sb[:, kc, pair * 128:(pair + 1) * 128],
                                xT_sb[:, kc, t4 * 512:(t4 + 1) * 512],
                                start=(kc == 0),
                                stop=(kc == 7),
                            )
                        nc.vector.tensor_copy(
                            dst[:, t4 * 512:(t4 + 1) * 512], ps[:]
                        )

            def alloc_qk(pair):
                qt = qkvpool.tile([128, T], BF16, tag=f"qT{pair}",
                                  name=f"qT{pair}")
                kt = qkvpool.tile([128, T], BF16, tag=f"kT{pair}",
                                  name=f"kT{pair}")
                return qt, kt

            # Software pipeline: x DMA and the next rep's pair-0 q/k are
            # issued one rep ahead so PE never starves at rep boundaries.
            xT_cur = xpool.tile([128, 8, T], BF16, tag="xT", name="xT0")
            emit_x_dma(xT_cur)
            qk0_cur = alloc_qk(0)
            emit_qk(0, xT_cur, *qk0_cur)

            for _rep in range(nrep):
                xT_sb = xT_cur
                qTs, kTs = {}, {}
                qTs[0], kTs[0] = qk0_cur
                # V' per (t-block, head): 64 cols of V then a ones column.
                # Split into 4 t-quarters so attention chunk c only gates on
                # the quarters it reads (per-tensor sems), not the whole V
                # phase.
                vp_sb = [
                    qkvpool.tile([128, 4, NH, HD + 1], BF16, tag=f"vp{q}",
                                 name=f"vp{q}")
                    for q in range(4)
                ]
                oT_sb = qkvpool.tile([128, 2, T], BF16, tag="oT", name="oT")
                for q in range(4):
                    nc.vector.memset(vp_sb[q][:, :, :, HD], 1.0)

                def emit_v(q):
                    # V in [t on partitions, head channels] layout, all 4
                    # heads, t-quarter q (t-blocks 4q..4q+3)
                    for tb in range(4 * q, 4 * q + 4):
                        ps = psA.tile([128, 512], F32, tag="aux", name="ps_v")
                        for kc in range(8):
                            nc.tensor.matmul(
                                ps[:, 0:CO],
                                xT_sb[:, kc, tb * 128:(tb + 1) * 128],
                                wv_sb[:, kc, :],
                                start=(kc == 0),
                                stop=(kc == 7),
                            )
                        nc.vector.tensor_copy(
                            vp_sb[tb // 4][:, tb % 4, :, 0:HD],
                            ps[:, 0:CO].rearrange("p (h d) -> p h d", h=NH),
                        )

                def attention_chunk(pair, c):
                    # queries [512c, 512c+512), key blocks jb = 0..4c+3
                    jb_last = 4 * c + 3
                    pos = [
                        psP.tile([65, 512], F32, tag="pos", name="pos")
                        for _ in range(2)
                    ]

                    def emit_s(jb):
                        off = 128 * max(0, jb - 4 * c)
                        ps_s = psS.tile([128, 1024], F32, tag="S", name="psS")
                        for hi in range(2):
                            nc.tensor.matmul(
                                ps_s[:, 512 * hi + off:512 * hi + 512],
                                kTs[pair][64 * hi:64 * hi + 64,
                                          jb * 128:(jb + 1) * 128],
                                qTs[pair][64 * hi:64 * hi + 64,
                                          c * 512 + off:(c + 1) * 512],
                                start=True,
                                stop=True,
                            )
                        et = etpool.tile([128, 1024], BF16, tag="et",
                                         name="et")
                        if off == 0:
                            nc.scalar.activation(et[:], ps_s[:], EXP,
                                                 scale=SCALE)
                        else:
                            # one strided activation covering both heads'
                            # valid spans [off:512) and [512+off:1024)
                            nc.scalar.activation(
                                et.rearrange("p (h w) -> p h w", h=2)[
                                    :, :, off:512],
                                ps_s.rearrange("p (h w) -> p h w", h=2)[
                                    :, :, off:512],
                                EXP, scale=SCALE,
                            )
                        if jb >= 4 * c:  # diagonal block: causal mask
                            for hi in range(2):
                                sl = slice(512 * hi + off, 512 * hi + off + 128)
                                nc.vector.tensor_tensor(
                                    et[:, sl], et[:, sl], mask_sb[:], MULT
                                )
                        return et, off

                    def emit_pv(jb, et, off):
                        for hi in range(2):
                            nc.tensor.matmul(
                                pos[hi][:, off:512],
                                vp_sb[jb // 4][:, jb % 4, 2 * pair + hi, :],
                                et[:, 512 * hi + off:512 * hi + 512],
                                start=(jb == 0),
                                stop=(jb == jb_last),
                            )

                    pending = emit_s(0)
                    for jb in range(jb_last + 1):
                        nxt = emit_s(jb + 1) if jb < jb_last else None
                        emit_pv(jb, *pending)
                        pending = nxt

                    # normalize: copy numerators out and take the recip of the
                    # sums row immediately (releases the pos PSUM for the next
                    # chunk's PV), then broadcast and scale oT in-place (all
                    # bf16 -> 2x DVE mode)
                    isl = slice(c * 512, (c + 1) * 512)
                    for hi in range(2):
                        o_sl = oT_sb[64 * hi:64 * hi + 64, pair, isl]
                        # ACT copy: keeps the latency-critical numerator move
                        # off the DVE queue (which carries qk/V/y copies)
                        nc.scalar.copy(o_sl, pos[hi][0:64, :])
                        rc = rcpool.tile([128, 512], BF16, tag="rc", name="rc")
                        nc.vector.reciprocal(rc[64:65, :], pos[hi][64:65, :])
                        bc = bcpool.tile([128, 512], BF16, tag="bc", name="bc")
                        # Pool-queue DMA: keeps the tiny broadcast off both
                        # the SP queue (x prefetch) and the ACT queue (exps).
                        # bc lands on the same partition range as o_sl: the
                        # verifier requires equal base partitions for SB+SB
                        # tensor_tensor operands.
                        bc_sl = bc[64 * hi:64 * hi + 64, :]
                        nc.gpsimd.dma_start(
                            bc_sl,
                            rc[64:65, None, :].to_broadcast([1, 64, 512]),
                        )
                        nc.vector.tensor_tensor(o_sl, o_sl, bc_sl, MULT)

                def outproj(c):
                    # t-blocks 4c..4c+3; one y DMA per 2 t-blocks
                    for g in range(2):
                        y2 = ypool.tile([128, 2, C], BF16, tag="yt", name="y2")
                        for sub in range(2):
                            tb = 4 * c + 2 * g + sub
                            for nk in range(2):
                                py = psA.tile([128, 512], F32, tag="aux",
                                              name="py")
                                for pr in range(2):
                                    nc.tensor.matmul(
                                        py[:],
                                        oT_sb[:, pr, tb * 128:(tb + 1) * 128],
                                        wo_sb[:, pr, nk * 512:(nk + 1) * 512],
                                        start=(pr == 0),
                                        stop=(pr == 1),
                                    )
                                dst = y2[:, sub, nk * 512:(nk + 1) * 512]
                                nc.vector.tensor_copy(dst, py[:])
                        tb2 = 2 * c + g
                        # Pool SWDGE queue: y stores don't block SP's x
                        # prefetch or the ACT queue's bc broadcasts
                        nc.gpsimd.dma_start(
                            y[tb2 * 256:(tb2 + 1) * 256, :].rearrange(
                                "(b p) c -> p b c", p=128
                            ),
                            y2[:],
                        )

                if _rep + 1 < nrep:
                    xT_cur = xpool.tile([128, 8, T], BF16, tag="xT",
                                        name="xTn")
                    emit_x_dma(xT_cur)
                for c in range(4):
                    emit_v(c)
                    attention_chunk(0, c)
                qTs[1], kTs[1] = alloc_qk(1)
                emit_qk(1, xT_sb, qTs[1], kTs[1])
                for c in range(3):
                    attention_chunk(1, c)
                    outproj(c)
                attention_chunk(1, 3)
                if _rep + 1 < nrep:
                    # next rep's pair-0 q/k ahead of the final out-proj: PE
                    # filler for the last norm-chain wait
                    qk0_cur = alloc_qk(0)
                    emit_qk(0, xT_cur, *qk0_cur)
                outproj(3)

    nc.compile()
    return nc


def _get_nc():
    global _compiled
    if _compiled is None:
        _compiled = _build()
    return _compiled


class _Runner:
    """Compiled PJRT executor for the SPMD kernel, reusable across calls."""

    def __init__(self, nc):
        import jax
        import concourse.mybir as mybir
        from concourse import bass2jax
        from jax.experimental.shard_map import shard_map
        from jax.sharding import Mesh, PartitionSpec

        self.jax = jax
        self.nc = nc
        bass2jax.install_neuronx_cc_hook()

        partition_name = (
            nc.partition_id_tensor.name if nc.partition_id_tensor else None
        )
        in_names, out_names, out_avals, zero_outs = [], [], [], []
        for alloc in nc.m.functions[0].allocations:
            if not isinstance(alloc, mybir.MemoryLocationSet):
                continue
            name = alloc.memorylocations[0].name
            if alloc.kind == "ExternalInput":
                if name != partition_name:
                    in_names.append(name)
            elif alloc.kind == "ExternalOutput":
                out_names.append(name)
                shape = tuple(alloc.tensor_shape)
                dtype = mybir.dt.np(alloc.dtype)
                out_avals.append(jax.core.ShapedArray(shape, dtype))
                zero_outs.append(np.zeros(shape, dtype))
        self.in_names = in_names
        self.out_names = out_names
        self.out_avals = out_avals
        self.zero_outs = zero_outs
        all_names = tuple(in_names + out_names)

        if partition_name is not None:
            all_names = all_names + (partition_name,)

        def _body(*args):
            operands = list(args)
            if partition_name is not None:
                operands.append(bass2jax.partition_id_tensor())
            outs = bass2jax._bass_exec_p.bind(
                *operands,
                out_avals=tuple(out_avals),
                in_names=all_names,
                out_names=tuple(out_names),
                lowering_input_output_aliases=(),
                sim_require_finite=True,
                sim_require_nnan=True,
                nc=nc,
            )
            return tuple(outs)

        devices = jax.devices()[:NCORES]
        assert len(devices) == NCORES
        mesh = Mesh(np.asarray(devices), ("core",))
        self._sharding = jax.sharding.NamedSharding(mesh, PartitionSpec("core"))
        n_args = len(in_names) + len(out_names)
        self.fn = jax.jit(
            shard_map(
                _body,
                mesh=mesh,
                in_specs=(PartitionSpec("core"),) * n_args,
                out_specs=(PartitionSpec("core"),) * len(out_names),
                check_rep=False,
            ),
            keep_unused=True,
        )

    def device_args(self, in_maps):
        args = [
            np.concatenate([np.asarray(m[name]) for m in in_maps], axis=0)
            for name in self.in_names
        ]
        args += [
            np.zeros((NCORES * z.shape[0], *z.shape[1:]), z.dtype)
            for z in self.zero_outs
        ]
        return [self.jax.device_put(a, self._sharding) for a in args]

    def run_device(self, dev_args):
        return self.fn(*dev_args)

    def run(self, in_maps):
        out_arrs = self.fn(*self.device_args(in_maps))
        return [
            {
                name: np.asarray(out_arrs[i]).reshape(
                    NCORES, *self.out_avals[i].shape
                )[c]
                for i, name in enumerate(self.out_names)
            }
            for c in range(NCORES)
        ]


_runner = None


def _get_runner():
    global _runner
    if _runner is None:
        _runner = _Runner(_get_nc())
    return _runner


def make_in_maps(x, Wqkv, Wo):
    bf16 = ml_dtypes.bfloat16
    x = np.asarray(x, dtype=np.float32)
    Wqkv = np.asarray(Wqkv, dtype=np.float32).astype(bf16)
    Wo = np.asarray(Wo, dtype=np.float32).astype(bf16)
    mask = np.triu(np.ones((128, 128), dtype=np.float32)).astype(bf16)
    in_maps = []
    for c in range(NCORES):
        b, g = c // 4, c % 4
        in_maps.append({
            "xT": np.ascontiguousarray(x[b].T.astype(bf16)),
            "wq": np.ascontiguousarray(Wqkv[:, g * CO:(g + 1) * CO]),
            "wk": np.ascontiguousarray(Wqkv[:, C + g * CO:C + (g + 1) * CO]),
            "wv": np.ascontiguousarray(Wqkv[:, 2 * C + g * CO:2 * C + (g + 1) * CO]),
            "wo": np.ascontiguousarray(Wo[g * CO:(g + 1) * CO, :]),
            "mask": mask,
        })
    return in_maps


def gather_output(results):
    y = np.zeros((B, T, C), dtype=np.float32)
    for c in range(NCORES):
        y[c // 4] += np.asarray(results[c]["y"]).astype(np.float32)
    return y


def kernel(x, Wqkv, Wo):
    runner = _get_runner()
    in_maps = make_in_maps(x, Wqkv, Wo)
    return gather_output(runner.run(in_maps))
